# revision 1
# baseline (speedup 1.0000x reference)
"""Trainium2 Bass kernel for nn_ActionEncoder (moe_routing).

Algorithm
---------
The module routes each of B=16384 samples to one of two small MLPs by
action_type, where the MLP input is a concatenation of one-hot vectors of
at most two indices in [0, 50).  Consequently there are only
50 (type 0) + 50*50 (type 1) = 2550 distinct possible outputs.

Instead of running the MLPs per sample (2 x [B,2550]x[2550,2550] matmuls),
each core:
  1. builds hidden vectors H for ALL distinct keys via one small matmul
     (one-hot pair mask) + relu,
  2. computes its 320-column slice of the 2550-row output TABLE
     (key -> trinary(h @ W2^T + b2)); output columns are sharded over the
     8 cores, so each core only streams a [2550, 320] weight slice.  The
     fp8 table ({-1,0,1} exact) stays resident in SBUF; the heavy matmul
     runs as fp8 DoubleRow (2 hidden k-tiles per pass),
  3. expands per-sample rows out[b] = table[key[b]] with one-hot routing
     masks: samples are grouped by table m-tile (128 keys) into 512-sample
     sub-chunks, each expanded by 4 PE matmuls (mask.T @ table_tile) whose
     f32 PSUM result is drained (DVE/ACT alternating) to bf16 and DMAed
     to the output ({-1,0,1} stays exact).  A final "overflow"
     chunk spanning all m-tiles absorbs samples beyond any chunk capacity,
     making the static graph safe for any type/key distribution.

Host work is restricted to layout marshalling: transposes/pads of weights,
the static one-hot pair mask, per-sample one-hot routing masks, and
re-assembly (row permutation + column concat) of the per-core outputs.

Numerics: H and W2 are stored fp8-e4m3, matmuls accumulate in f32.  With
the reference's 0.02 weight scale every pre-activation satisfies
|y| < ~0.2 and the fp8 path error is < ~0.01 -- far from the trinary
thresholds at +-0.5, so no trinary output can flip.  The expansion
matmuls (one-hot x {-1,0,1}) and all post-trinary dtypes are exact.
"""

import os
import sys

import numpy as np

if "/opt/trn_rl_repo" not in sys.path:
    sys.path.insert(0, "/opt/trn_rl_repo")

# ---- problem constants (hardcoded per harness spec) ----
B = 16384          # batch
HID = 2550         # N_PRED (hidden and output width)
HIDP = 2560        # hidden padded to 20*128
NKH = HIDP // 128  # 20 hidden k-tiles
NCORE = 8
QS = 320           # output-column slice per core (8*320 = 2560 >= 2550)
T1_BASE = 128      # first slot of type-1 keys (m-tile aligned)
NSLOT = 2688       # 21 * 128 key slots
NMT = NSLOT // 128
SUB = 512          # samples per expansion sub-chunk
OV = 128           # overflow-chunk capacity

_NC_CACHE = {}


def plan_chunks(b):
    """Static expansion plan: m-tile served by each regular sub-chunk."""
    t0_subs = -(-(b // 2 + SUB) // SUB)  # type-0 capacity: b/2 + 512
    return [0] * t0_subs + list(range(1, NMT))


def out_rows(b):
    return len(plan_chunks(b)) * SUB + OV


def build_nc(b=B):
    """Build the (single, SPMD) Bass graph; identical on all 8 cores."""
    import concourse.bacc as bacc
    import concourse.bass as bass
    import concourse.mybir as mybir
    import concourse.tile as tile

    FP = mybir.dt.float32
    BF = mybir.dt.bfloat16
    F8 = mybir.dt.float8e4
    AF = mybir.ActivationFunctionType
    OP = mybir.AluOpType

    mts = plan_chunks(b)
    nsub = len(mts)
    nrows = out_rows(b)

    nc = bacc.Bacc(None, target_bir_lowering=False)

    w1tb = nc.declare_dram_parameter("w1tb", [128, HIDP], FP, isOutput=False)
    oh = nc.declare_dram_parameter("oh", [128, 2500], BF, isOutput=False)
    w10 = nc.declare_dram_parameter("w10", [HIDP, 50], FP, isOutput=False)
    b10 = nc.declare_dram_parameter("b10", [128, NKH], FP, isOutput=False)
    w2ta = nc.declare_dram_parameter("w2ta", [HIDP, QS], FP, isOutput=False)
    w2tb = nc.declare_dram_parameter("w2tb", [HIDP, QS], FP, isOutput=False)
    masks = nc.declare_dram_parameter("masks", [nsub, 128, SUB], F8, isOutput=False)
    omask = nc.declare_dram_parameter("omask", [128, NMT + 1, OV], F8, isOutput=False)
    out_e = nc.declare_dram_parameter("out", [nrows, QS], F8, isOutput=True)

    with tile.TileContext(nc) as tc:
        with (
            tc.tile_pool(name="const", bufs=1) as const,
            tc.tile_pool(name="stg", bufs=3) as stg,
            tc.tile_pool(name="hp", bufs=1) as hp,
            tc.tile_pool(name="psp", bufs=3, space=bass.MemorySpace.PSUM) as psp,
            tc.tile_pool(name="tri", bufs=2) as tri,
            tc.tile_pool(name="msk", bufs=3) as msk,
        ):
            # ---- PE warm-up: dep-free matmuls un-throttle the HAM clock
            # gate (cold 1.2 GHz -> warm 2.4 GHz) before real work arrives
            wu_t = const.tile([128, 256], BF)
            nc.vector.memset(wu_t[:], 0.0)
            for _ in range(24):
                psw = psp.tile([128, 256], FP, tag="pw", bufs=1)
                nc.tensor.matmul(
                    psw[:], wu_t[:, 0:128], wu_t[:], start=True, stop=True
                )

            # ---- load constants ----
            oh_t = const.tile([128, 2500], BF)
            nc.sync.dma_start(out=oh_t[:], in_=oh[:, :])
            b10_t = const.tile([128, NKH], FP)
            nc.sync.dma_start(out=b10_t[:], in_=b10[:, :])
            w10_t = const.tile([128, NKH, 50], FP)
            nc.sync.dma_start(
                out=w10_t[:], in_=w10[:, :].rearrange("(k p) i -> p k i", p=128)
            )
            w1tb_s = stg.tile([128, HIDP], FP, tag="w1stg", bufs=1)
            nc.sync.dma_start(out=w1tb_s[:], in_=w1tb[:, :])
            w1tb_b = const.tile([128, HIDP], BF)
            nc.vector.tensor_copy(w1tb_b[:], w1tb_s[:])

            w2a_b = const.tile([128, NKH, QS], F8)
            w2b_b = const.tile([128, NKH, QS], F8)
            for src, dst in ((w2ta, w2a_b), (w2tb, w2b_b)):
                s = stg.tile([128, NKH, QS], FP, tag="w2stg", bufs=2)
                nc.sync.dma_start(
                    out=s[:], in_=src[:, :].rearrange("(k p) q -> p k q", p=128)
                )
                nc.vector.tensor_copy(dst[:], s[:])

            # bf16 table, SBUF-resident; zero first (pad rows stay 0)
            tab = const.tile([128, NMT + 1, QS], F8)
            nc.vector.memset(tab[:], 0.0)

            # ---- H: hidden vectors for all key slots ----
            h_b = hp.tile([128, NKH, NSLOT], F8)
            for k in range(NKH):
                # type-0 keys: h = relu(W1_0[:, i] + b1_0)  (DVE: add + max0)
                nc.vector.tensor_scalar(
                    h_b[:, k, 0:50],
                    w10_t[:, k, :],
                    b10_t[:, k : k + 1],
                    0.0,
                    OP.add,
                    OP.max,
                )
            for cp in ((0, 1), (2, 3), (4,)):
                # type-1 keys: h = relu(W1_1^T rows (i, 50+j) summed + b1_1)
                for k in range(NKH):
                    ps2 = psp.tile([128, 2, 512], FP, tag="ps2", bufs=2)
                    for i, c in enumerate(cp):
                        nc.tensor.matmul(
                            ps2[:, i, 0:500],
                            w1tb_b[:, k * 128 : (k + 1) * 128],
                            oh_t[:, c * 500 : (c + 1) * 500],
                            start=True,
                            stop=True,
                        )
                    lo = T1_BASE + cp[0] * 500
                    wid = 1000 if len(cp) == 2 else 500
                    src_ap = ps2[:, :, 0:500] if len(cp) == 2 else ps2[:, 0, 0:500]
                    nc.scalar.activation(h_b[:, k, lo : lo + wid], src_ap, AF.Relu)
            # (the H[hidden=2550,:]=1 bias-trick row is produced by the relu
            # paths themselves: host sets b10[2550]=1 and w1tb[100,2550]=1)

            # ---- table: tab[:, mk, :] = trinary(H[:, slot]^T @ W2T) ----
            mtiles = [(0, 50, "a")]
            for mk in range(1, NMT):
                mtiles.append((mk * 128, 128 if mk < NMT - 1 else 68, "b"))
            for mk, (c0, m, which) in enumerate(mtiles):
                pst = psp.tile([128, QS], FP, tag="ps", bufs=3)
                w2t = w2a_b if which == "a" else w2b_b
                for t in range(NKH // 2):
                    nc.tensor.matmul(
                        pst[0:m, :],
                        h_b[:, 2 * t : 2 * t + 2, c0 : c0 + m],
                        w2t[:, 2 * t : 2 * t + 2, :],
                        start=(t == 0),
                        stop=(t == NKH // 2 - 1),
                        perf_mode=mybir.MatmulPerfMode.DoubleRow,
                    )
                bm_t = tri.tile([128, QS], FP, tag="tb")
                nc.vector.tensor_scalar(
                    bm_t[0:m, :], pst[0:m, :], -0.5, -1.0, OP.is_ge, OP.add
                )
                nc.vector.scalar_tensor_tensor(
                    tab[0:m, mk, :], pst[0:m, :], 0.5, bm_t[0:m, :], OP.is_gt, OP.add
                )

            # ---- expansion: out[pos, :] = table[key(pos), :] ----
            out_v = out_e[: nsub * SUB, :].rearrange(
                "(s j p) q -> s p j q", j=SUB // 128, p=128
            )
            for g0 in range(0, nsub, 4):
                gn = min(4, nsub - g0)
                mk_t = msk.tile([128, 4, SUB], F8, tag="mk", bufs=10)
                nc.scalar.dma_start(
                    out=mk_t[:, 0:gn, :],
                    in_=masks[g0 : g0 + gn].rearrange("s p m -> p s m"),
                )
                for si in range(gn):
                    s = g0 + si
                    mt = mts[s]
                    oc_t = msk.tile([128, SUB // 128, QS], F8, tag="oc", bufs=6)
                    if s % 2 == 0:
                        for jp in range(SUB // 256):
                            pp = psp.tile([128, 2, 512], FP, tag="ps2", bufs=2)
                            for i in range(2):
                                j = 2 * jp + i
                                nc.tensor.matmul(
                                    pp[:, i, 0:QS],
                                    mk_t[:, si, j * 128 : (j + 1) * 128],
                                    tab[:, mt, :],
                                    start=True,
                                    stop=True,
                                )
                                nc.vector.tensor_copy(
                                    oc_t[:, j, :], pp[:, i, 0:QS]
                                )
                    else:
                        for j in range(SUB // 128):
                            pse = psp.tile([128, QS], FP, tag="ps", bufs=3)
                            nc.tensor.matmul(
                                pse[:],
                                mk_t[:, si, j * 128 : (j + 1) * 128],
                                tab[:, mt, :],
                                start=True,
                                stop=True,
                            )
                            nc.scalar.activation(oc_t[:, j, :], pse[:], AF.Copy)
                    nc.sync.dma_start(out=out_v[s], in_=oc_t[:])
            # overflow chunk: spans all m-tiles
            om_t = msk.tile([128, NMT + 1, OV], F8, tag="om", bufs=1)
            nc.sync.dma_start(out=om_t[:], in_=omask[:, :, :])
            ov_v = out_e[nsub * SUB :, :].rearrange("(j p) q -> p j q", p=128)
            oo_t = msk.tile([128, OV // 128, QS], F8, tag="oo", bufs=1)
            npair = (NMT + 1) // 2
            for j in range(OV // 128):
                psoT = psp.tile([128, QS], FP, tag="ps", bufs=3)
                pso = psoT[:]
                for t in range(npair):
                    nc.tensor.matmul(
                        pso,
                        om_t[:, 2 * t : 2 * t + 2, j * 128 : (j + 1) * 128],
                        tab[:, 2 * t : 2 * t + 2, :],
                        start=(t == 0),
                        stop=(t == npair - 1),
                        perf_mode=mybir.MatmulPerfMode.DoubleRow,
                    )
                nc.vector.tensor_copy(oo_t[:, j, :], pso)
            nc.sync.dma_start(out=ov_v[:], in_=oo_t[:])

    nc.compile()
    return nc


def route(inputs, b):
    """Host routing: slot per sample, sample -> device output row."""
    ai = np.asarray(inputs["action_indices"]).astype(np.int64)
    at = np.asarray(inputs["action_types"]).astype(np.int64)
    i0, i1 = ai[:b, 0], ai[:b, 1]
    slot = np.where(at[:b] == 0, i0, T1_BASE + i0 * 50 + i1).astype(np.int64)

    mts = plan_chunks(b)
    nsub = len(mts)
    serves = {}
    for s, mt in enumerate(mts):
        serves.setdefault(mt, []).append(s)
    fill = np.zeros(nsub + 1, dtype=np.int64)
    pos = np.empty(b, dtype=np.int64)  # device out row per sample
    chunk_of = np.empty(b, dtype=np.int64)
    for i in range(b):
        mt = slot[i] >> 7
        for s in serves[mt]:
            if fill[s] < SUB:
                chunk_of[i], pos[i] = s, s * SUB + fill[s]
                fill[s] += 1
                break
        else:  # overflow chunk
            assert fill[nsub] < OV, "overflow chunk exhausted"
            chunk_of[i], pos[i] = nsub, nsub * SUB + fill[nsub]
            fill[nsub] += 1
    return slot, chunk_of, pos, mts, nsub


def marshal(inputs, b=B):
    """Host-side layout marshalling -> per-core input maps + row map."""
    import ml_dtypes

    F8 = ml_dtypes.float8_e4m3
    W1_0 = np.asarray(inputs["W1_0"], dtype=np.float32)
    b1_0 = np.asarray(inputs["b1_0"], dtype=np.float32)
    W2_0 = np.asarray(inputs["W2_0"], dtype=np.float32)
    b2_0 = np.asarray(inputs["b2_0"], dtype=np.float32)
    W1_1 = np.asarray(inputs["W1_1"], dtype=np.float32)
    b1_1 = np.asarray(inputs["b1_1"], dtype=np.float32)
    W2_1 = np.asarray(inputs["W2_1"], dtype=np.float32)
    b2_1 = np.asarray(inputs["b2_1"], dtype=np.float32)

    slot, chunk_of, pos, mts, nsub = route(inputs, b)

    # routing masks: one-hot (key within m-tile) x (position in chunk)
    masks = np.zeros((nsub, 128, SUB), dtype=F8)
    omask = np.zeros((128, NMT + 1, OV), dtype=F8)
    reg = chunk_of < nsub
    s_r, i_r = chunk_of[reg], np.flatnonzero(reg)
    masks[s_r, slot[i_r] - (np.asarray(mts)[s_r] << 7), pos[i_r] - s_r * SUB] = 1
    i_o = np.flatnonzero(~reg)
    omask[slot[i_o] & 127, slot[i_o] >> 7, pos[i_o] - nsub * SUB] = 1

    oh = np.zeros((128, 2500), dtype=ml_dtypes.bfloat16)
    kk = np.arange(2500)
    oh[kk // 50, kk] = 1
    oh[50 + kk % 50, kk] = 1
    oh[100, :] = 1  # b1_1 bias row

    w1tb = np.zeros((128, HIDP), dtype=np.float32)
    w1tb[:100, :HID] = W1_1.T
    w1tb[100, :HID] = b1_1
    w1tb[100, HID] = 1.0  # bias-trick: makes H[2550, type-1 slots] = 1

    w10 = np.zeros((HIDP, 50), dtype=np.float32)
    w10[:HID] = W1_0
    b10 = np.zeros(HIDP, dtype=np.float32)
    b10[:HID] = b1_0
    b10[HID] = 1.0  # bias-trick: makes H[2550, type-0 slots] = 1
    b10 = np.ascontiguousarray(b10.reshape(NKH, 128).T)  # [128, NKH]

    W2T0 = np.ascontiguousarray(W2_0.T)  # W2T[h, q] = W2[q, h]
    W2T1 = np.ascontiguousarray(W2_1.T)

    shared = {
        "w1tb": w1tb,
        "oh": oh,
        "w10": w10,
        "b10": b10,
        "masks": masks,
        "omask": omask,
    }
    in_maps = []
    for k in range(NCORE):
        qlo = k * QS
        w = max(0, min(HID - qlo, QS))
        w2ta = np.zeros((HIDP, QS), dtype=np.float32)
        w2tb = np.zeros((HIDP, QS), dtype=np.float32)
        w2ta[:HID, :w] = W2T0[:, qlo : qlo + w]
        w2ta[HID, :w] = b2_0[qlo : qlo + w]
        w2tb[:HID, :w] = W2T1[:, qlo : qlo + w]
        w2tb[HID, :w] = b2_1[qlo : qlo + w]
        in_maps.append(dict(shared, w2ta=w2ta, w2tb=w2tb))
    return in_maps, pos


def unshard(outs, pos, b=B):
    """Per-core column slices + row map -> [b, 2550] float32."""
    parts = []
    for k in range(NCORE):
        qlo = k * QS
        w = max(0, min(HID - qlo, QS))
        parts.append(np.asarray(outs[k])[:, :w])
    rows = np.concatenate(parts, axis=1)
    return np.ascontiguousarray(rows[pos], dtype=np.float32)


def kernel(**inputs):
    from concourse.bass_utils import run_bass_kernel_spmd

    if "nc" not in _NC_CACHE:
        _NC_CACHE["nc"] = build_nc()
    nc = _NC_CACHE["nc"]
    in_maps, pos = marshal(inputs)
    trace = bool(int(os.environ.get("BASSK_TRACE", "0")))
    res = run_bass_kernel_spmd(nc, in_maps, core_ids=list(range(NCORE)), trace=trace)
    _NC_CACHE["last_results"] = res
    return unshard([res.results[k]["out"] for k in range(NCORE)], pos)

